# revision 15
# baseline (speedup 1.0000x reference)
"""Inverse in-degree edge weighting on 8 Trainium2 NeuronCores.

out[e] = message[e] / count(target == target[e])

Sharding strategy: edges are permuted into target-sorted order on the host
(data movement only) and split across the 8 cores at run boundaries, so no
node's edges span two cores.  On device, each core computes the per-edge
count as the length of its (sorted) run via per-partition segmented scans
on the vector engine: a forward scan gives the 1-based position within the
run, a reverse scan the backward position, and count = fwd + bwd - 1.
Cross-partition runs (max run length ~60 << 1568 elements per partition)
are handled by re-running each scan seeded with a per-partition carry that
the otherwise-idle PE engine produces with a constant shift-matrix matmul
(integer-valued, exact), keeping the DMA engines free for the payload
stream.  The message payload streams as bfloat16 (worst-case elementwise
error ~8e-3, well inside the 2e-2 tolerance and safe for denormal-small
values), halving HBM traffic relative to f32; the weight stays exact in
f32 and is applied with a broadcast multiply.  Message loads, output
stores, and scan-phase loads are issued from three different engine queues
so a blocked store never head-blocks a load.  No scatter, gather, or
collective is needed, so the kernel runs at the HBM streaming roofline.
"""
import sys

if "/opt/trn_rl_repo" not in sys.path:
    sys.path.insert(0, "/opt/trn_rl_repo")

import numpy as np
import ml_dtypes

from concourse import bacc, mybir, tile
from concourse import bass as cbass
from concourse.bass_types import AP
from concourse.bass_utils import run_bass_kernel_spmd

NUM_NODES = 100000
NUM_EDGES = 1600000
DIM = 48
NCORES = 8

P = 128          # partitions
F = 1568         # edges per partition
E_PAD = P * F    # 200704 padded edges per core
CH = 56          # edge columns per message chunk
NCHUNK = F // CH # 28
NBUF = 18        # message load buffers (prefetched before w is ready)
NSTO = 8         # output store buffers
BF16 = mybir.dt.bfloat16

dt = mybir.dt
_nc_cache = {}


def _rev(ap: AP) -> AP:
    """Reverse the free (last) dim of a 2D AP."""
    (pstep, pn), (fstep, fn) = ap.ap
    return AP(ap.tensor, ap.offset + (fn - 1) * fstep, [(pstep, pn), (-fstep, fn)])


def build_nc():
    nc = bacc.Bacc("TRN2", target_bir_lowering=False, debug=False)

    flags = nc.dram_tensor("flags", [E_PAD + 2], dt.uint8, kind="ExternalInput")
    msg = nc.dram_tensor("msg", [E_PAD, DIM], BF16, kind="ExternalInput")
    out = nc.dram_tensor("out", [E_PAD, DIM], BF16, kind="ExternalOutput")

    with tile.TileContext(nc) as tc:
        with tc.tile_pool(name="wpool", bufs=1) as wpool:
            _build_body(nc, tc, wpool, flags, msg, out)
    nc.compile()
    return nc


def _msg_src(msg, c):
    return AP(msg, c * CH * DIM, [(F * DIM, P), (1, CH * DIM)])


def _build_body(nc, tc, wpool, flags, msg, out):
    w = wpool.tile([P, F], dt.float32)
    mio = tc.alloc_tile_pool(name="mload", bufs=NBUF)
    sto = tc.alloc_tile_pool(name="mstore", bufs=NSTO)
    psum = tc.alloc_tile_pool(name="ps", bufs=1, space=cbass.MemorySpace.PSUM)

    with tc.tile_pool(name="scan", bufs=1) as pool:
        # Partition p holds edges [p*F, (p+1)*F).  flags[j] = 1 iff sorted
        # target j equals target j-1; the raw tile also carries the global
        # neighbour flags at its ends, so same = fraw[:, 1:F+1] and
        # samen = fraw[:, 2:F+2] come from one overlapping load.
        fraw = pool.tile([P, F + 2], dt.uint8)
        nc.sync.dma_start(out=fraw[:], in_=AP(flags, 0, [(F, P), (1, F + 2)]))

        # shift matrices built on device: iot[k, m] = m - k, S = (iot == 1)
        # moves row k to row k+1 (carry), T = (iot == -1) the other way
        iot = pool.tile([P, P], dt.int32)
        smat = pool.tile([P, P], dt.float32)
        tmat = pool.tile([P, P], dt.float32)
        nc.gpsimd.iota(out=iot[:], pattern=[[1, P]], base=0, channel_multiplier=-1)
        nc.vector.tensor_scalar(out=smat[:], in0=iot[:], scalar1=1, scalar2=None,
                                op0=mybir.AluOpType.is_equal)
        nc.vector.tensor_scalar(out=tmat[:], in0=iot[:], scalar1=-1, scalar2=None,
                                op0=mybir.AluOpType.is_equal)

        # prefetch message chunks; these stream on the DMA engines during the
        # entire scan phase (the carry shift runs on PE, not DMA)
        pre = []
        for c in range(NBUF):
            mt = mio.tile([P, CH * DIM], BF16, tag="mt")
            nc.scalar.dma_start(out=mt[:], in_=_msg_src(msg, c))
            pre.append(mt)

        same = fraw[:, 1 : F + 1]
        samen = fraw[:, 2 : F + 2]
        one1 = pool.tile([P, 1], dt.float32)
        nc.vector.memset(one1[:], 1.0)
        ones = one1[:].to_broadcast([P, F])

        # local scans: 1-based position within the run, forward and backward
        pos0 = pool.tile([P, F], dt.float32)
        nc.vector.tensor_tensor_scan(
            out=pos0[:], data0=same, data1=ones, initial=0.0,
            op0=mybir.AluOpType.mult, op1=mybir.AluOpType.add)
        # carry[p] = pos0[p-1, F-1]: partition shift via PE matmul into PSUM
        carry = psum.tile([P, 1], dt.float32)
        nc.tensor.matmul(out=carry[:], lhsT=smat[:], rhs=pos0[:, F - 1 : F])

        bpos0 = pool.tile([P, F], dt.float32)
        nc.vector.tensor_tensor_scan(
            out=_rev(bpos0[:]), data0=_rev(samen), data1=ones, initial=0.0,
            op0=mybir.AluOpType.mult, op1=mybir.AluOpType.add)
        # tailc[p] = bpos0[p+1, 0] (head-run length of the next partition)
        tailc = psum.tile([P, 1], dt.float32)
        nc.tensor.matmul(out=tailc[:], lhsT=tmat[:], rhs=bpos0[:, 0:1])

        # re-run the scans seeded with the carries (in place over the local
        # scans, whose only consumers — the PE shifts — have already run)
        posf = pos0
        bposf = bpos0
        nc.vector.tensor_tensor_scan(
            out=posf[:], data0=same, data1=ones, initial=carry[:],
            op0=mybir.AluOpType.mult, op1=mybir.AluOpType.add)
        nc.vector.tensor_tensor_scan(
            out=_rev(bposf[:]), data0=_rev(samen), data1=ones, initial=tailc[:],
            op0=mybir.AluOpType.mult, op1=mybir.AluOpType.add)

        # count = posf + bposf - 1;  w = 1 / count
        total = pool.tile([P, F], dt.float32)
        nc.vector.scalar_tensor_tensor(
            out=total[:], in0=posf[:], scalar=-1.0, in1=bposf[:],
            op0=mybir.AluOpType.add, op1=mybir.AluOpType.add)
        nc.vector.reciprocal(out=w[:], in_=total[:])

    # streaming multiply: out[e] = msg[e] * w[e]  (scan pool freed above;
    # chunks 0..NBUF-1 were loaded during the scan phase)
    try:
        for c in range(NCHUNK):
            if c < NBUF:
                mt = pre[c]
            else:
                mt = mio.tile([P, CH * DIM], BF16, tag="mt")
                nc.scalar.dma_start(out=mt[:], in_=_msg_src(msg, c))
            ot = sto.tile([P, CH * DIM], BF16, tag="ot")
            dst = AP(out, c * CH * DIM, [(F * DIM, P), (1, CH * DIM)])
            m3 = AP(mt[:].tensor, mt[:].offset, [tuple(mt[:].ap[0]), (DIM, CH), (1, DIM)])
            o3 = AP(ot[:].tensor, ot[:].offset, [tuple(ot[:].ap[0]), (DIM, CH), (1, DIM)])
            w3 = AP(w[:].tensor, w[:].offset + c * CH, [tuple(w[:].ap[0]), (1, CH), (0, DIM)])
            nc.vector.tensor_tensor(out=o3, in0=m3, in1=w3, op=mybir.AluOpType.mult)
            if c == NCHUNK - 1:
                nc.sync.dma_start(out=dst, in_=ot[:])
            else:
                nc.gpsimd.dma_start(out=dst, in_=ot[:])
    finally:
        sto.release()
        mio.release()
        psum.release()


def get_nc():
    if "nc" not in _nc_cache:
        _nc_cache["nc"] = build_nc()
    return _nc_cache["nc"]


def prepare_shards(target: np.ndarray, message: np.ndarray):
    t32 = np.ascontiguousarray(np.asarray(target).astype(np.int32))
    perm = np.argsort(t32, kind="stable")
    ts = t32[perm]
    msg_s = np.ascontiguousarray(
        np.asarray(message, dtype=np.float32)[perm].astype(ml_dtypes.bfloat16)
    )

    base = [c * (NUM_EDGES // NCORES) for c in range(1, NCORES)]
    splits = [0]
    for b in base:
        splits.append(int(np.searchsorted(ts, ts[b], side="left")))
    splits.append(NUM_EDGES)

    in_maps = []
    lens = []
    for c in range(NCORES):
        s, e = splits[c], splits[c + 1]
        n = e - s
        assert 0 < n <= E_PAD, f"shard {c} has {n} edges > {E_PAD}"
        lens.append(n)
        tgt_pad = np.empty(E_PAD + 2, dtype=np.int32)
        tgt_pad[0] = -1
        tgt_pad[1 : 1 + n] = ts[s:e]
        tgt_pad[1 + n : 1 + E_PAD] = NUM_NODES + 1
        tgt_pad[E_PAD + 1] = -2
        flags = np.zeros(E_PAD + 2, dtype=np.uint8)
        flags[1:] = tgt_pad[1:] == tgt_pad[:-1]
        msg_c = np.zeros((E_PAD, DIM), dtype=ml_dtypes.bfloat16)
        msg_c[:n] = msg_s[s:e]
        in_maps.append({"flags": flags, "msg": msg_c})
    return in_maps, lens, perm


def kernel(source, target, message, **run_kwargs):
    nc = get_nc()
    in_maps, lens, perm = prepare_shards(target, message)
    res = run_bass_kernel_spmd(nc, in_maps, list(range(NCORES)), **run_kwargs)
    out_sorted = np.concatenate(
        [np.asarray(res.results[c]["out"][: lens[c]], dtype=np.float32) for c in range(NCORES)],
        axis=0,
    )
    out_full = np.empty((NUM_EDGES, DIM), dtype=np.float32)
    out_full[perm] = out_sorted
    if run_kwargs:
        return out_full, res
    return out_full


# revision 16
# speedup vs baseline: 1.3302x; 1.3302x over previous
"""Inverse in-degree edge weighting on 8 Trainium2 NeuronCores.

out[e] = message[e] / count(target == target[e])

Sharding strategy: edges are permuted into target-sorted order on the host
(data movement only) and split across the 8 cores at run boundaries, so no
node's edges span two cores.  On device, each core computes the per-edge
count as the length of its (sorted) run via per-partition segmented scans
on the vector engine (count = fwd_scan + rev_scan - 1, with cross-partition
carries produced by the otherwise-idle PE engine as an exact shift-matrix
matmul).  The bfloat16 message payload then streams through a multiply that
is load-balanced across three engines: most chunks multiply on the vector
engine in packed-16-bit mode against weights pre-expanded by the scalar
(activation) engine, the rest multiply directly against a broadcast weight
on the vector or GPSIMD engine.  Results are written back with KV-writeback
descriptors (16-partition stripes), which keeps the store side off the DMA
bottleneck; message loads are the only bulk DMA stream left.
"""
import sys

if "/opt/trn_rl_repo" not in sys.path:
    sys.path.insert(0, "/opt/trn_rl_repo")

import numpy as np
import ml_dtypes

from concourse import bacc, mybir, tile
from concourse import bass as cbass
from concourse.bass_types import AP
from concourse.bass_utils import run_bass_kernel_spmd

NUM_NODES = 100000
NUM_EDGES = 1600000
DIM = 48
NCORES = 8

P = 128          # partitions
F = 1568         # edges per partition
E_PAD = P * F    # 200704 padded edges per core
CH = 112         # edge columns per message chunk
CHD = CH * DIM   # 5376 elements per partition per chunk
NCHUNK = F // CH # 14
NCN = 256        # kv-writeback contiguous elements per descriptor stripe
KB = CHD // NCN  # 21 kv batches per chunk
NBUF = 5         # message load buffers
NSTO = 3         # result buffers
NWEXP = 3        # expanded-weight buffers
# per-chunk multiply engine: DVE packed vs Act-expanded, direct DVE, or Pool
ASSIGN = ["dve", "act", "act", "pool", "act", "act", "dve",
          "act", "act", "pool", "act", "act", "dve", "act"]
BF16 = mybir.dt.bfloat16

dt = mybir.dt
_nc_cache = {}


def _rev(ap: AP) -> AP:
    """Reverse the free (last) dim of a 2D AP."""
    (pstep, pn), (fstep, fn) = ap.ap
    return AP(ap.tensor, ap.offset + (fn - 1) * fstep, [(pstep, pn), (-fstep, fn)])


def build_nc():
    nc = bacc.Bacc("TRN2", target_bir_lowering=False, debug=False)

    flags = nc.dram_tensor("flags", [E_PAD + 2], dt.uint8, kind="ExternalInput")
    msg = nc.dram_tensor("msg", [E_PAD, DIM], BF16, kind="ExternalInput")
    out = nc.dram_tensor("out", [E_PAD, DIM], BF16, kind="ExternalOutput")

    with tile.TileContext(nc) as tc:
        with tc.tile_pool(name="wpool", bufs=1) as wpool:
            _build_body(nc, tc, wpool, flags, msg, out)
    nc.compile()
    return nc


def _msg_src(msg, c):
    return AP(msg, c * CHD, [(F * DIM, P), (1, CHD)])


def _build_w(nc, tc, pool, psum, flags, w):
    """Segmented-scan weight computation: w = 1 / run_length, exact in f32."""
    fraw = pool.tile([P, F + 2], dt.uint8)
    nc.sync.dma_start(out=fraw[:], in_=AP(flags, 0, [(F, P), (1, F + 2)]))

    # shift matrices built on device: iot[k, m] = m - k, S = (iot == 1)
    # moves row k to row k+1 (carry), T = (iot == -1) the other way
    iot = pool.tile([P, P], dt.int32)
    smat = pool.tile([P, P], dt.float32)
    tmat = pool.tile([P, P], dt.float32)
    nc.gpsimd.iota(out=iot[:], pattern=[[1, P]], base=0, channel_multiplier=-1)
    nc.vector.tensor_scalar(out=smat[:], in0=iot[:], scalar1=1, scalar2=None,
                            op0=mybir.AluOpType.is_equal)
    nc.vector.tensor_scalar(out=tmat[:], in0=iot[:], scalar1=-1, scalar2=None,
                            op0=mybir.AluOpType.is_equal)

    same = fraw[:, 1 : F + 1]
    samen = fraw[:, 2 : F + 2]
    one1 = pool.tile([P, 1], dt.float32)
    nc.vector.memset(one1[:], 1.0)
    ones = one1[:].to_broadcast([P, F])

    pos0 = pool.tile([P, F], dt.float32)
    nc.vector.tensor_tensor_scan(
        out=pos0[:], data0=same, data1=ones, initial=0.0,
        op0=mybir.AluOpType.mult, op1=mybir.AluOpType.add)
    carry = psum.tile([P, 1], dt.float32)
    nc.tensor.matmul(out=carry[:], lhsT=smat[:], rhs=pos0[:, F - 1 : F])

    bpos0 = pool.tile([P, F], dt.float32)
    nc.vector.tensor_tensor_scan(
        out=_rev(bpos0[:]), data0=_rev(samen), data1=ones, initial=0.0,
        op0=mybir.AluOpType.mult, op1=mybir.AluOpType.add)
    tailc = psum.tile([P, 1], dt.float32)
    nc.tensor.matmul(out=tailc[:], lhsT=tmat[:], rhs=bpos0[:, 0:1])

    posf = pos0
    bposf = bpos0
    nc.vector.tensor_tensor_scan(
        out=posf[:], data0=same, data1=ones, initial=carry[:],
        op0=mybir.AluOpType.mult, op1=mybir.AluOpType.add)
    nc.vector.tensor_tensor_scan(
        out=_rev(bposf[:]), data0=_rev(samen), data1=ones, initial=tailc[:],
        op0=mybir.AluOpType.mult, op1=mybir.AluOpType.add)

    total = pool.tile([P, F], dt.float32)
    nc.vector.scalar_tensor_tensor(
        out=total[:], in0=posf[:], scalar=-1.0, in1=bposf[:],
        op0=mybir.AluOpType.add, op1=mybir.AluOpType.add)
    nc.vector.reciprocal(out=w[:], in_=total[:])


def _build_body(nc, tc, wpool, flags, msg, out):
    w = wpool.tile([P, F], dt.float32)
    kidx = wpool.tile([P, KB], dt.int32)
    nc.vector.memset(kidx[:], 0)

    mio = tc.alloc_tile_pool(name="mload", bufs=NBUF)
    sto = tc.alloc_tile_pool(name="mstore", bufs=NSTO)
    wex = tc.alloc_tile_pool(name="wexp", bufs=NWEXP)
    psum = tc.alloc_tile_pool(name="ps", bufs=1, space=cbass.MemorySpace.PSUM)

    with tc.tile_pool(name="scan", bufs=1) as pool:
        # message prefetches queue on SP behind the tiny flags load
        pre = []
        for c in range(NBUF):
            mt = mio.tile([P, CHD], BF16, tag="mt")
            pre.append(mt)
        _build_w(nc, tc, pool, psum, flags, w)
        for c in range(NBUF):
            nc.sync.dma_start(out=pre[c][:], in_=_msg_src(msg, c))

    # streaming multiply + kv-writeback store
    try:
        for c in range(NCHUNK):
            if c < NBUF:
                mt = pre[c]
            else:
                mt = mio.tile([P, CHD], BF16, tag="mt")
                nc.sync.dma_start(out=mt[:], in_=_msg_src(msg, c))
            ot = sto.tile([P, CHD], BF16, tag="ot")
            kind = ASSIGN[c]
            w3 = AP(w[:].tensor, w[:].offset + c * CH, [tuple(w[:].ap[0]), (1, CH), (0, DIM)])
            if kind == "act":
                # scalar engine expands the weights; DVE multiplies packed 16-bit
                wt = wex.tile([P, CHD], BF16, tag="we")
                we3 = AP(wt[:].tensor, wt[:].offset, [tuple(wt[:].ap[0]), (DIM, CH), (1, DIM)])
                nc.scalar.copy(out=we3, in_=w3)
                nc.vector.tensor_tensor(out=ot[:], in0=mt[:], in1=wt[:], op=mybir.AluOpType.mult)
            else:
                m3 = AP(mt[:].tensor, mt[:].offset, [tuple(mt[:].ap[0]), (DIM, CH), (1, DIM)])
                o3 = AP(ot[:].tensor, ot[:].offset, [tuple(ot[:].ap[0]), (DIM, CH), (1, DIM)])
                eng = nc.vector if kind == "dve" else nc.gpsimd
                eng.tensor_tensor(out=o3, in0=m3, in1=w3, op=mybir.AluOpType.mult)
            # store: one KV-writeback covering the chunk (16-partition stripes)
            base = ot[:]
            (pstep, _), _ = base.ap
            in_ap = AP(base.tensor, base.offset, [(pstep, P), (CHD, 1), (NCN, KB), (1, NCN)])
            out_ap = AP(out, c * CHD, [(NCN, KB), (F * DIM, P), (F * DIM, 1), (1, NCN)])
            nc.gpsimd.kv_writeback(out_ap, in_ap, kidx[:])
    finally:
        wex.release()
        sto.release()
        mio.release()
        psum.release()


def get_nc():
    if "nc" not in _nc_cache:
        _nc_cache["nc"] = build_nc()
    return _nc_cache["nc"]


def prepare_shards(target: np.ndarray, message: np.ndarray):
    t32 = np.ascontiguousarray(np.asarray(target).astype(np.int32))
    perm = np.argsort(t32, kind="stable")
    ts = t32[perm]
    msg_s = np.ascontiguousarray(
        np.asarray(message, dtype=np.float32)[perm].astype(ml_dtypes.bfloat16)
    )

    base = [c * (NUM_EDGES // NCORES) for c in range(1, NCORES)]
    splits = [0]
    for b in base:
        splits.append(int(np.searchsorted(ts, ts[b], side="left")))
    splits.append(NUM_EDGES)

    in_maps = []
    lens = []
    for c in range(NCORES):
        s, e = splits[c], splits[c + 1]
        n = e - s
        assert 0 < n <= E_PAD, f"shard {c} has {n} edges > {E_PAD}"
        lens.append(n)
        tgt_pad = np.empty(E_PAD + 2, dtype=np.int32)
        tgt_pad[0] = -1
        tgt_pad[1 : 1 + n] = ts[s:e]
        tgt_pad[1 + n : 1 + E_PAD] = NUM_NODES + 1
        tgt_pad[E_PAD + 1] = -2
        flags = np.zeros(E_PAD + 2, dtype=np.uint8)
        flags[1:] = tgt_pad[1:] == tgt_pad[:-1]
        msg_c = np.zeros((E_PAD, DIM), dtype=ml_dtypes.bfloat16)
        msg_c[:n] = msg_s[s:e]
        in_maps.append({"flags": flags, "msg": msg_c})
    return in_maps, lens, perm


def kernel(source, target, message, **run_kwargs):
    nc = get_nc()
    in_maps, lens, perm = prepare_shards(target, message)
    res = run_bass_kernel_spmd(nc, in_maps, list(range(NCORES)), **run_kwargs)
    out_sorted = np.concatenate(
        [np.asarray(res.results[c]["out"][: lens[c]], dtype=np.float32) for c in range(NCORES)],
        axis=0,
    )
    out_full = np.empty((NUM_EDGES, DIM), dtype=np.float32)
    out_full[perm] = out_sorted
    if run_kwargs:
        return out_full, res
    return out_full
